# revision 9
# baseline (speedup 1.0000x reference)
"""Trainium2 Bass kernel for BasisDecorrelationLoss.

Math: per sample b, with x = depth_basis[b] ([C=32, N=76800]) and mask m ([N]):
    mu_c  = (1/N) sum_n x[c,n]                      (unmasked spatial mean)
    S_cd  = sum_n x[c,n] x[d,n] m[n]                (masked Gram, the heavy part)
    t_c   = sum_n x[c,n] m[n]
    M     = sum_n m[n]
    cov   = (S - mu t^T - t mu^T + mu mu^T M) / M   (mean-centered masked covariance)
    zncc  = clamp(cov,eps) / (sigma sigma^T), loss_b = mean(zncc^2)
    loss  = mean_b loss_b

Device strategy (data-parallel, one sample per NeuronCore, 8 cores):
  The host concatenates x and m into one [33, N] tensor per sample. SBUF uses a
  "slab" layout: partition p holds n in [p*600, (p+1)*600) as 33 strips of
  contiguous floats (contiguous DMA runs, no transposes, one DMA per chunk).
  One augmented matmul accumulation computes S, t, sum_x and M at once: per
  contraction step j, weights lhsT = [X_j*m | 1] (33 cols, strided AP) and
  moving rhs = [X_j | m_j | 1] (34 cols) accumulate into PSUM [33, 34] over 600
  K=128 steps:
      out[c<32, d<32] = S,  out[c<32, 33] = t,  out[32, d<32] = N*mu,
      out[32, 32] = M,      out[32, 33] = N.
  The mask multiply is one DVE tensor_tensor per chunk with a stride-0
  broadcast AP. Host does the final [32,32] math and averages the 8 per-sample
  scalars (the "scalar all-reduce").
"""

import numpy as np

import concourse.bacc as bacc
import concourse.bass as bass
import concourse.tile as tile
import concourse.tile_rust as tile_rust
from concourse import mybir
from concourse.bass_utils import run_bass_kernel_spmd

B = 8
C = 32
H, W = 240, 320
N = H * W            # 76800
P = 128              # SBUF partitions
NPP = N // P         # 600 n-values per partition
NQ = 4               # chunks (pipeline depth)
JC = NPP // NQ       # contraction steps per chunk
EPS = 1e-10

_F32 = mybir.dt.float32
_BF16 = mybir.dt.bfloat16


def _build_kernel_body(tc: "tile.TileContext", xm_d: bass.AP, out_d: bass.AP):
    nc = tc.nc

    # n = p*NPP + q*JC + j ; rows 0..31 = x, row 32 = m
    xm_view = xm_d.rearrange("c (p q j) -> q p c j", p=P, q=NQ, j=JC)

    with (
        tc.tile_pool(name="slabs", bufs=2) as slabs,
        tc.tile_pool(name="psum", bufs=1, space="PSUM") as psum,
        tc.tile_pool(name="outp", bufs=1) as outp,
    ):
        acc = psum.tile([C + 1, C + 2], _F32)
        junk = psum.tile([1, 1], _F32)

        for q in range(NQ):
            # fp32 landing slab straight from HBM (full-rate 600B runs)
            s32_t = slabs.tile([P, C + 1, JC], _F32)
            nc.sync.dma_start(out=s32_t, in_=xm_view[q])

            # bf16 stream slab: strips 0..31 = x, 32 = m (ACT cast), 33 = ones
            s_t = slabs.tile([P, C + 2, JC], _BF16)
            nc.scalar.copy(out=s_t[:, 0 : C + 1, :], in_=s32_t)
            nc.vector.memset(s_t[:, C + 1, :], 1.0)

            # bf16 weights slab: strips 0..31 = x*m, strip 32 = ones
            w_t = slabs.tile([P, C + 1, JC], _BF16)
            nc.vector.memset(w_t[:, C, :], 1.0)

            # w[:, 0:C, :] = s[:, 0:C, :] * m  (mask broadcast across strips)
            m_strip = s_t[:, C, :]
            m_bc = bass.AP(
                tensor=m_strip.tensor,
                offset=m_strip.offset,
                ap=[m_strip.ap[0], [0, C], m_strip.ap[1]],
            )
            nc.vector.tensor_mul(w_t[:, 0:C, :], s_t[:, 0:C, :], m_bc)

            # Sync-carrier matmul: walrus gives LDWEIGHTS a single sync-wait
            # slot, but the first real matmul of a chunk would need two (ACT
            # for s_t + DVE for w_t). This 1x1 matmul reads only ACT-written
            # strips, consuming the ACT wait on PE so the real matmuls only
            # ever carry the DVE wait.
            sync_mm = nc.tensor.matmul(
                junk[:, :],
                lhsT=s_t[:, C, 0:1],
                rhs=s_t[:, C, 0:1],
                start=True,
                stop=True,
            )

            first_mm = None
            for j in range(JC):
                mm = nc.tensor.matmul(
                    acc[:, :],
                    lhsT=w_t[:, :, j],
                    rhs=s_t[:, :, j],
                    start=(q == 0 and j == 0),
                    stop=(q == NQ - 1 and j == JC - 1),
                )
                if first_mm is None:
                    first_mm = mm
                    tile_rust.add_dep_helper(
                        first_mm.ins, sync_mm.ins, sync=False,
                        reason="sync-carrier before real matmuls",
                    )

        res = outp.tile([C + 1, C + 2], _F32)
        nc.any.tensor_copy(res, acc)
        nc.sync.dma_start(out=out_d, in_=res)


def _build_nc() -> bass.Bass:
    nc = bacc.Bacc()
    xm = nc.declare_dram_parameter("xm", [C + 1, N], _F32, isOutput=False)
    out = nc.declare_dram_parameter("out", [C + 1, C + 2], _F32, isOutput=True)
    with tile.TileContext(nc) as tc:
        _build_kernel_body(tc, xm[:], out[:])
    nc.finalize()
    return nc


def _finalize(gathered: list[np.ndarray]) -> np.ndarray:
    """Host-side per-sample [33,34] -> scalar loss, averaged over batch."""
    total = 0.0
    for G in gathered:
        G = G.astype(np.float64)
        S = G[0:C, 0:C]
        t = G[0:C, C + 1]
        sx = G[C, 0:C]
        M = G[C, C]
        mu = sx / N
        cov = (S - np.outer(mu, t) - np.outer(t, mu) + np.outer(mu, mu) * M) / M
        cov = np.maximum(cov, EPS)
        sig = np.sqrt(np.diag(cov))
        zncc = cov / np.outer(sig, sig)
        total += float(np.mean(zncc * zncc))
    return np.array(total / B, dtype=np.float32)


_NC_CACHE = None


def _run(depth_basis: np.ndarray, mask: np.ndarray, trace: bool = False):
    global _NC_CACHE
    if _NC_CACHE is None:
        _NC_CACHE = _build_nc()
    nc = _NC_CACHE

    x_full = np.asarray(depth_basis, dtype=np.float32).reshape(B, C, N)
    m_full = np.asarray(mask, dtype=np.float32).reshape(B, 1, N)
    xm_full = np.ascontiguousarray(np.concatenate([x_full, m_full], axis=1))

    in_maps = [{"xm": xm_full[i]} for i in range(B)]
    r = run_bass_kernel_spmd(nc, in_maps, list(range(B)), trace=trace)
    gathered = [np.asarray(r.results[i]["out"]) for i in range(B)]
    return _finalize(gathered), r


def kernel(depth_basis: np.ndarray, mask: np.ndarray) -> np.ndarray:
    loss, _ = _run(depth_basis, mask, trace=False)
    return loss


# revision 12
# speedup vs baseline: 1.2051x; 1.2051x over previous
"""Trainium2 Bass kernel for BasisDecorrelationLoss.

Math: per sample b, with x = depth_basis[b] ([C=32, N=76800]) and mask m ([N]):
    mu_c  = (1/N) sum_n x[c,n]                      (unmasked spatial mean)
    S_cd  = sum_n x[c,n] x[d,n] m[n]                (masked Gram, the heavy part)
    t_c   = sum_n x[c,n] m[n]
    M     = sum_n m[n]
    cov   = (S - mu t^T - t mu^T + mu mu^T M) / M   (mean-centered masked covariance)
    zncc  = clamp(cov,eps) / (sigma sigma^T), loss_b = mean(zncc^2)
    loss  = mean_b loss_b

Device strategy (data-parallel, one sample per NeuronCore, 8 cores):
  The host concatenates x and m into one [33, N] tensor per sample. SBUF uses a
  "slab" layout: partition p holds n in [p*600, (p+1)*600) as 33 strips of
  contiguous floats (contiguous DMA runs, no transposes, one DMA per chunk).
  One augmented matmul accumulation computes S, t, sum_x and M at once: per
  contraction step j, weights lhsT = [X_j*m | 1] (33 cols, strided AP) and
  moving rhs = [X_j | m_j | 1] (34 cols) accumulate into PSUM [33, 34] over 600
  K=128 steps:
      out[c<32, d<32] = S,  out[c<32, 33] = t,  out[32, d<32] = N*mu,
      out[32, 32] = M,      out[32, 33] = N.
  The mask multiply is one DVE tensor_tensor per chunk with a stride-0
  broadcast AP. Host does the final [32,32] math and averages the 8 per-sample
  scalars (the "scalar all-reduce").
"""

import numpy as np

import concourse.bacc as bacc
import concourse.bass as bass
import concourse.tile as tile
import concourse.tile_rust as tile_rust
from concourse import mybir
from concourse.bass_utils import run_bass_kernel_spmd

B = 8
C = 32
H, W = 240, 320
N = H * W            # 76800
P = 128              # SBUF partitions
NPP = N // P         # 600 n-values per partition
NQ = 4               # chunks (pipeline depth)
JC = NPP // NQ       # contraction steps per chunk
EPS = 1e-10

_F32 = mybir.dt.float32
_BF16 = mybir.dt.bfloat16


NG = 3               # col-groups used for the Gram (j mod NG)
JB = 15              # stats j-block: 33*15 = 495 <= 512 moving cols
NJB = JC // JB


def _build_kernel_body(tc: "tile.TileContext", xm_d: bass.AP, out_d: bass.AP,
                       out2_d: bass.AP):
    nc = tc.nc

    # n = p*NPP + q*JC + j ; rows 0..31 = x, row 32 = m
    xm_view = xm_d.rearrange("c (p q j) -> q p c j", p=P, q=NQ, j=JC)

    with (
        tc.tile_pool(name="slabs", bufs=2) as slabs,
        tc.tile_pool(name="psum", bufs=1, space="PSUM") as psum,
        tc.tile_pool(name="outp", bufs=1) as outp,
    ):
        # NG blocks of [32, 34]: block g accumulates Gram+t over j = g (mod NG)
        acc = psum.tile([NG * C, C + 2], _F32)
        # stats row: [1, 33*JB]; col (c*JB + jj) = sum over (p, q, jb) of
        # strip c at j = jb*JB + jj  ->  host sums the JB columns per strip
        sacc_full = psum.tile([P, (C + 1) * JB], _F32)
        sacc = sacc_full[96:97, :]

        for q in range(NQ):
            # fp32 landing slab straight from HBM (full-rate 600B runs)
            s32_t = slabs.tile([P, C + 1, JC], _F32)
            nc.sync.dma_start(out=s32_t, in_=xm_view[q])

            # bf16 stream slab: strips 0..31 = x, 32 = m (ACT cast), 33 = ones
            s_t = slabs.tile([P, C + 2, JC], _BF16)
            nc.scalar.copy(out=s_t[:, 0 : C + 1, :], in_=s32_t)
            nc.vector.memset(s_t[:, C + 1, :], 1.0)

            # bf16 weights slab: 32 strips of x*m (mask broadcast across strips)
            w_t = slabs.tile([P, C, JC], _BF16)
            m_strip = s_t[:, C, :]
            m_bc = bass.AP(
                tensor=m_strip.tensor,
                offset=m_strip.offset,
                ap=[m_strip.ap[0], [0, C], m_strip.ap[1]],
            )
            nc.vector.tensor_mul(w_t, s_t[:, 0:C, :], m_bc)

            # Stats matmuls in col-group 3: ones.T @ [X | m] in wide blocks.
            # Scheduled before the Gram matmuls, they also consume the ACT
            # cast wait on PE, so the first Gram matmul only carries the DVE
            # wait (walrus allows a single sync-wait per LDWEIGHTS).
            last_stats = None
            for jb in range(NJB):
                sm = nc.tensor.matmul(
                    sacc[:, :],
                    lhsT=s_t[:, C + 1, 0:1],
                    rhs=s_t[:, 0 : C + 1, jb * JB : (jb + 1) * JB],
                    start=(q == 0 and jb == 0),
                    stop=(q == NQ - 1 and jb == NJB - 1),
                    tile_position=(0, 96),
                )
                last_stats = sm

            first_mm = None
            for j in range(JC):
                g = (q * JC + j) % NG
                mm = nc.tensor.matmul(
                    acc[32 * g : 32 * (g + 1), :],
                    lhsT=w_t[:, :, j],
                    rhs=s_t[:, :, j],
                    start=(q == 0 and j < NG),
                    stop=(q == NQ - 1 and j >= JC - NG),
                    tile_position=(0, 32 * g),
                )
                if first_mm is None:
                    first_mm = mm
                    tile_rust.add_dep_helper(
                        mm.ins, last_stats.ins, sync=False,
                        reason="stats matmuls drain the ACT wait first",
                    )

        res = outp.tile([NG * C, C + 2], _F32)
        nc.any.tensor_copy(res, acc)
        nc.sync.dma_start(out=out_d, in_=res)
        res2 = outp.tile([1, (C + 1) * JB], _F32)
        nc.any.tensor_copy(res2, sacc)
        nc.sync.dma_start(out=out2_d, in_=res2)


def _build_nc() -> bass.Bass:
    nc = bacc.Bacc()
    xm = nc.declare_dram_parameter("xm", [C + 1, N], _F32, isOutput=False)
    out = nc.declare_dram_parameter("out", [NG * C, C + 2], _F32, isOutput=True)
    out2 = nc.declare_dram_parameter("out2", [1, (C + 1) * JB], _F32,
                                     isOutput=True)
    with tile.TileContext(nc) as tc:
        _build_kernel_body(tc, xm[:], out[:], out2[:])
    nc.finalize()
    return nc


def _finalize(gathered: list[tuple[np.ndarray, np.ndarray]]) -> np.ndarray:
    """Host-side per-sample ([96,34], [1,495]) -> scalar loss, batch mean."""
    total = 0.0
    for G, G2 in gathered:
        G = G.astype(np.float64)
        S = np.zeros((C, C))
        t = np.zeros(C)
        for g in range(NG):
            S += G[32 * g : 32 * (g + 1), 0:C]
            t += G[32 * g : 32 * (g + 1), C + 1]
        stats = G2.astype(np.float64).reshape(C + 1, JB).sum(axis=1)
        mu = stats[0:C] / N
        M = stats[C]
        cov = (S - np.outer(mu, t) - np.outer(t, mu) + np.outer(mu, mu) * M) / M
        cov = np.maximum(cov, EPS)
        sig = np.sqrt(np.diag(cov))
        zncc = cov / np.outer(sig, sig)
        total += float(np.mean(zncc * zncc))
    return np.array(total / B, dtype=np.float32)


_NC_CACHE = None


def _run(depth_basis: np.ndarray, mask: np.ndarray, trace: bool = False):
    global _NC_CACHE
    if _NC_CACHE is None:
        _NC_CACHE = _build_nc()
    nc = _NC_CACHE

    x_full = np.asarray(depth_basis, dtype=np.float32).reshape(B, C, N)
    m_full = np.asarray(mask, dtype=np.float32).reshape(B, 1, N)
    xm_full = np.ascontiguousarray(np.concatenate([x_full, m_full], axis=1))

    in_maps = [{"xm": xm_full[i]} for i in range(B)]
    r = run_bass_kernel_spmd(nc, in_maps, list(range(B)), trace=trace)
    gathered = [
        (np.asarray(r.results[i]["out"]), np.asarray(r.results[i]["out2"]))
        for i in range(B)
    ]
    return _finalize(gathered), r


def kernel(depth_basis: np.ndarray, mask: np.ndarray) -> np.ndarray:
    loss, _ = _run(depth_basis, mask, trace=False)
    return loss
